# revision 27
# baseline (speedup 1.0000x reference)
"""Trainium2 Bass kernel for ActivationSparseLinear (batched GEMV).

out[b, 0, n] = sum_k x[b, 0, k] * weight[n, k]
  x: (8, 1, 4096) f32, weight: (11008, 4096) f32 -> out: (8, 1, 11008) f32

Strategy (tensor-parallel over out_features, 8 NeuronCores):
  - Each core owns 1408 rows of `weight` (slightly overlapping shards so
    every core sees 11 full 128-row tiles) and the full (tiny) `x`.
  - Memory-bound on the f32 weight stream (~23 MB/core).  The weight is
    DMA'd with an on-the-fly f32->bf16 cast (SWDGE), transposed on the
    TensorEngine via identity matmuls (k onto partitions), bounced
    PSUM->SBUF on DVE/ACT (casting to bf16), then consumed as the
    512-wide moving operand of bf16 matmuls whose stationary operand is
    the 8-column x^T tile, accumulating f32 [8, 512] in PSUM.
  - A burst of dummy matmuls at kernel start (while the first weight DMA
    is in flight) warms the PE HAM clock-gate from 1.2 to 2.4 GHz.
  - No cross-core communication; the host concatenates the 8 shards.
"""

from contextlib import ExitStack

import numpy as np

import concourse.bass as bass
import concourse.bacc as bacc
import concourse.mybir as mybir
import concourse.tile as tile
from concourse.bass_utils import run_bass_kernel_spmd

B = 8          # batch (seq_len 1 folded away)
K = 4096       # in_features
N = 11008      # out_features
NCORES = 8
N_SHARD = 1408                 # rows per core (8*1376=11008; shards overlap)
KT = K // 128                  # 32 k-tiles
NCHUNK = 512                   # output rows per psum accumulator chunk
KSEG = 1024                    # k columns per weight DMA segment (2MB f32)
N_WARMUP = 40                  # dummy matmuls to warm the PE clock

# per-core shard start rows (core 7 shifted so shards stay in range)
SHARD_STARTS = [min(c * (N // NCORES), N - N_SHARD) for c in range(NCORES)]

_GRAPH_CACHE = {}


def build_graph() -> bacc.Bacc:
    nc = bacc.Bacc("TRN2", target_bir_lowering=False, debug=False,
                   num_devices=NCORES)
    w = nc.declare_dram_parameter("w", [N_SHARD, K], mybir.dt.float32,
                                  isOutput=False)
    xt = nc.declare_dram_parameter("xt", [128, KT * B], mybir.dt.bfloat16,
                                   isOutput=False)
    ident = nc.declare_dram_parameter("ident", [128, 128], mybir.dt.bfloat16,
                                      isOutput=False)
    out = nc.declare_dram_parameter("out", [N_SHARD, B], mybir.dt.float32,
                                    isOutput=True)

    bf16 = mybir.dt.bfloat16
    f32 = mybir.dt.float32

    chunks = []
    r = 0
    while r < N_SHARD:
        chunks.append((r, min(NCHUNK, N_SHARD - r)))
        r += NCHUNK

    with tile.TileContext(nc) as tc, ExitStack() as ctx:
        const_pool = ctx.enter_context(tc.tile_pool(name="const", bufs=1))
        wn_pool = ctx.enter_context(tc.tile_pool(name="wn", bufs=8))
        wt_pool = ctx.enter_context(tc.tile_pool(name="wt", bufs=6))
        pst_pool = ctx.enter_context(
            tc.tile_pool(name="pst", bufs=4, space="PSUM"))
        psa_pool = ctx.enter_context(
            tc.tile_pool(name="psa", bufs=1, space="PSUM"))
        out_pool = ctx.enter_context(tc.tile_pool(name="outp", bufs=2))

        # constants: x^T (host-pretransposed to [k_in_tile, kt*B]) and the
        # transpose identity, already bf16 on host; HWDGE load keeps the
        # gpsimd SWDGE queue free for the weight stream.
        xt_sb = const_pool.tile([128, KT * B], bf16)
        nc.sync.dma_start(xt_sb[:], xt[:])
        id_sb = const_pool.tile([128, 128], bf16)
        nc.sync.dma_start(id_sb[:], ident[:])

        n_copy = 0
        for row0, nrows in chunks:
            jn = nrows // 128
            # acc[p, j, b] accumulates out rows row0 + j*128 + p; each j
            # lives in its own PSUM bank — an accumulation group's
            # start=True clears has_written for its whole bank, so
            # concurrent groups must not share one
            acc_ps = psa_pool.tile([128, 4, 512], f32, tag="acc")

            # segment tiles: w_seg[p, j, kk] = w[row0 + j*128 + p, s*KSEG+kk]
            segs = []
            for s in range(K // KSEG):
                w_sb = wn_pool.tile([128, jn, KSEG], bf16, tag="w_sb")
                # two 1MB half-segment DMAs: finer completion granularity
                # lets transposes start as soon as the first half lands
                for h in range(2):
                    k0 = s * KSEG + h * (KSEG // 2)
                    src = w[row0:row0 + nrows, k0:k0 + KSEG // 2]
                    nc.gpsimd.dma_start(
                        w_sb[:, :, h * (KSEG // 2):(h + 1) * (KSEG // 2)],
                        src.rearrange("(j p) k -> p j k", p=128))
                segs.append(w_sb)

            for s, w_sb in enumerate(segs):
                for kk in range(KSEG // 128):
                    kt = s * (KSEG // 128) + kk
                    tp_ps = pst_pool.tile([128, NCHUNK], f32, tag="tp")
                    wt_sb = wt_pool.tile([128, NCHUNK], bf16, tag="wt")
                    for j in range(jn):
                        # transpose as a NORMAL matmul with identity rhs
                        # (out = w_tile.T @ I) so the PE's HAM clock-gate
                        # sees real matmul activity; bf16 weights also
                        # get the fast weight load.
                        nc.tensor.matmul(
                            tp_ps[:, j * 128:(j + 1) * 128],
                            w_sb[:, j, kk * 128:(kk + 1) * 128],
                            id_sb[:],
                        )
                    # the copy casts the exact f32 psum values to bf16
                    if n_copy % 2 == 0:
                        nc.vector.tensor_copy(wt_sb[:, :nrows],
                                              tp_ps[:, :nrows])
                    else:
                        nc.scalar.copy(wt_sb[:, :nrows], tp_ps[:, :nrows])
                    n_copy += 1
                    for j in range(jn):
                        # GEMV: W^T tile stationary (its LDWEIGHTS hides
                        # behind in-flight matmuls via the background
                        # weight buffer), 8-column x^T moving
                        nc.tensor.matmul(
                            acc_ps[:, j, :B],
                            wt_sb[:, j * 128:(j + 1) * 128],
                            xt_sb[:, kt * B:(kt + 1) * B],
                            start=(kt == 0),
                            stop=(kt == KT - 1),
                        )
            o_sb = out_pool.tile([128, 4 * B], f32, tag="o")
            nc.scalar.copy(
                o_sb[:, :jn * B].rearrange("p (j b) -> p j b", b=B),
                acc_ps[:, :jn, :B])
            nc.sync.dma_start(
                out[row0:row0 + nrows, :].rearrange("(j p) b -> p j b",
                                                    p=128),
                o_sb[:, :jn * B].rearrange("p (j b) -> p j b", b=B))

    nc.compile()
    return nc


def _get_graph() -> bacc.Bacc:
    if "nc" not in _GRAPH_CACHE:
        _GRAPH_CACHE["nc"] = build_graph()
    return _GRAPH_CACHE["nc"]


def _make_in_maps(x: np.ndarray, weight: np.ndarray):
    x = np.asarray(x, dtype=np.float32).reshape(B, K)
    weight = np.asarray(weight, dtype=np.float32)
    bf16_np = mybir.dt.np(mybir.dt.bfloat16)
    # xt[p, kt*B + b] = x[b, kt*128 + p]
    xt = np.ascontiguousarray(
        x.reshape(B, KT, 128).transpose(2, 1, 0).reshape(128, KT * B)
    ).astype(bf16_np)
    ident = np.eye(128, dtype=np.float32).astype(bf16_np)
    in_maps = []
    for c in range(NCORES):
        s0 = SHARD_STARTS[c]
        w_shard = np.ascontiguousarray(weight[s0:s0 + N_SHARD, :])
        in_maps.append({"w": w_shard, "xt": xt, "ident": ident})
    return in_maps


def _run(x: np.ndarray, weight: np.ndarray, trace: bool = False):
    nc = _get_graph()
    in_maps = _make_in_maps(x, weight)
    res = run_bass_kernel_spmd(nc, in_maps, core_ids=list(range(NCORES)),
                               trace=trace)
    out = np.empty((B, 1, N), dtype=np.float32)
    base = N // NCORES
    for c in range(NCORES):
        lo = c * base                      # global start of unique range
        off = lo - SHARD_STARTS[c]         # offset inside this core's shard
        out[:, 0, lo:lo + base] = res.results[c]["out"][off:off + base, :].T
    return out, res


def kernel(x: np.ndarray, weight: np.ndarray) -> np.ndarray:
    out, _ = _run(x, weight, trace=False)
    return out
